# revision 1
# baseline (speedup 1.0000x reference)
"""Trainium2 Bass kernel for nn_CINLayer: out[b,d,o] = sum_{n,m} x[b,d,n]*y[b,d,m]*W[o,n*M+m].

Strategy (8-core data parallel over batch):
  Per sample s, out[o,s] = sum_k Wl[k,o] * Z[k,s] with Z[k,s] = x[s,n(k)]*y[s,m(k)].
  The contraction k (1600 products) is split into 13 chunks of 128 rows whose
  row->(n,m) mapping is chosen so each chunk's X-factor tile is a single
  DVE stream_shuffle of a host-staged interleaved layout Xil (per-quadrant
  lane-broadcast), and the Y-factor tiles are host-staged replicated layouts.
  Z chunks are built as one shuffle + one fp16 tensor_mul, then fed as the
  moving operand of fp16 matmuls accumulating out^T[o, s] in PSUM
  (o split 128+72, s tiles of 512).

  Chunk row mapping (r = 32j + r', j=quadrant):
    Part A (c<10):  (n, m) = (4c + j, r')          for r' < 32
    Part B (cb=c-10<3): r' = 8a + m''; (n, m) = (16cb + 4a + j, 32 + m'')
  Host layouts:
    Xil[32j + i]  = xT[4i + j]   (i<10, else 0)
    YrepA[p]      = yT[p % 32]
    YrepB[p]      = yT[32 + p % 8]
  Shuffle masks: A: mask[r'] = c ; B: mask[r'] = 4*cb + r'//8.
  W rows with n >= 40 (part B overhang) are zeroed on host.
"""

import numpy as np

BS, DIM, N, M, O = 2048, 32, 40, 40, 200
NCORES = 8
S_PER_CORE = BS * DIM // NCORES  # 8192
S_TILE = 512
N_STILES_FULL = S_PER_CORE // S_TILE  # 16
NCHUNKS = 13  # 10 part-A + 3 part-B
F16 = np.float16

# chunks whose Z-multiply runs on GPSIMD instead of DVE. GPSIMD's tensor_mul
# is ~9x slower per op than DVE's, but running a few there in parallel with
# the DVE shuffle/mul stream measured fastest (190us vs 214us all-DVE).
GPSIMD_MULS = frozenset({2, 4, 6, 9, 11})


def _chunk_row_to_nm(c: int, r: int):
    """Global chunk c (0..12), row r (0..127) -> (n, m) or None (zero pad)."""
    j, rp = divmod(r, 32)
    if c < 10:
        return 4 * c + j, rp
    cb = c - 10
    a, mpp = divmod(rp, 8)
    n = 16 * cb + 4 * a + j
    if n >= N:
        return None
    return n, 32 + mpp


def _shuffle_mask(c: int):
    if c < 10:
        return [c] * 32
    cb = c - 10
    return [4 * cb + (rp // 8) for rp in range(32)]


def _stage_w(W: np.ndarray) -> np.ndarray:
    """W [O, N*M] f32 -> wl [128, NCHUNKS, O] f16 (lhsT layout per chunk)."""
    Wr = W.reshape(O, N, M)
    wl = np.zeros((128, NCHUNKS, O), dtype=F16)
    for c in range(NCHUNKS):
        for r in range(128):
            nm = _chunk_row_to_nm(c, r)
            if nm is not None:
                wl[r, c, :] = Wr[:, nm[0], nm[1]].astype(F16)
    return wl


def _stage_core_inputs(x_flat: np.ndarray, y_flat: np.ndarray):
    """x_flat, y_flat [S_PER_CORE, 40] f32 -> xil, yrepa, yrepb [128, S] f16."""
    xT = np.ascontiguousarray(x_flat.T).astype(F16)  # [40, S]
    yT = np.ascontiguousarray(y_flat.T).astype(F16)  # [40, S]
    s = xT.shape[1]
    xil = np.zeros((128, s), dtype=F16)
    for p in range(128):
        j, i = divmod(p, 32)[0], p % 32
        if i < 10:
            xil[p] = xT[4 * i + j]
    yrepa = yT[np.arange(128) % 32]
    yrepb = yT[32 + (np.arange(128) % 8)]
    return xil, np.ascontiguousarray(yrepa), np.ascontiguousarray(yrepb)


def build_nc(n_stiles: int = N_STILES_FULL, debug: bool = False):
    """Build the per-core Bass/Tile module. Returns (nc, names dict)."""
    import concourse.bass as bass
    import concourse.tile as tile
    from concourse import bacc, mybir
    from concourse.tile_rust import add_dep_helper

    f16 = mybir.dt.float16
    f32 = mybir.dt.float32
    s_len = n_stiles * S_TILE

    nc = bacc.Bacc("TRN2", target_bir_lowering=False, debug=debug)

    xil_d = nc.dram_tensor("xil", [128, s_len], f16, kind="ExternalInput")
    ya_d = nc.dram_tensor("yrepa", [128, s_len], f16, kind="ExternalInput")
    yb_d = nc.dram_tensor("yrepb", [128, s_len], f16, kind="ExternalInput")
    wl_d = nc.dram_tensor("wl", [128, NCHUNKS, O], f16, kind="ExternalInput")
    out_d = nc.dram_tensor("outt", [O, s_len], f16, kind="ExternalOutput")

    with tile.TileContext(nc) as tc:
        with (
            tc.tile_pool(name="wpool", bufs=1) as wpool,
            tc.tile_pool(name="inp", bufs=4) as inp,
            tc.tile_pool(name="xe", bufs=8) as xep,
            tc.tile_pool(name="zp", bufs=8) as zp,
            tc.tile_pool(name="outp", bufs=4) as outp,
            tc.tile_pool(name="ps", bufs=2, space=bass.MemorySpace.PSUM) as psp,
        ):
            wl_sb = wpool.tile([128, NCHUNKS, O], f16)
            nc.sync.dma_start(wl_sb[:], wl_d[:])

            # Paired s-tiles: each shuffle/mul covers 1024 samples (two matmul
            # tiles) to halve DVE op count and PE supply-wait events; the four
            # PSUM accumulation chains use exactly 8 banks at bufs=2.
            W2 = 2 * S_TILE
            for t2 in range(n_stiles // 2):
                sl2 = bass.ts(t2, W2)
                xil_t = inp.tile([128, W2], f16)
                nc.sync.dma_start(xil_t[:], xil_d[:, sl2])
                ya_t = inp.tile([128, W2], f16)
                nc.sync.dma_start(ya_t[:], ya_d[:, sl2])
                yb_t = inp.tile([128, W2], f16)
                nc.sync.dma_start(yb_t[:], yb_d[:, sl2])

                psA0 = psp.tile([128, S_TILE], f32, tag="psA0")
                psB0 = psp.tile([72, S_TILE], f32, tag="psB0")
                psA1 = psp.tile([128, S_TILE], f32, tag="psA1")
                psB1 = psp.tile([72, S_TILE], f32, tag="psB1")
                ps = [psA0, psB0, psA1, psB1]
                for c in range(NCHUNKS):
                    xe = xep.tile([128, W2], f16, tag="xe")
                    nc.vector.stream_shuffle(xe[:], xil_t[:], _shuffle_mask(c))
                    z = zp.tile([128, W2], f16)
                    yt = ya_t if c < 10 else yb_t
                    eng = nc.gpsimd if c in GPSIMD_MULS else nc.vector
                    eng.tensor_mul(z[:], yt[:], xe[:])
                    first, last = c == 0, c == NCHUNKS - 1
                    for h in range(2):
                        zh = z[:, h * S_TILE : (h + 1) * S_TILE]
                        nc.tensor.matmul(
                            ps[2 * h][:], wl_sb[:, c, 0:128], zh,
                            start=first, stop=last,
                        )
                        nc.tensor.matmul(
                            ps[2 * h + 1][:], wl_sb[:, c, 128:200], zh,
                            start=first, stop=last,
                        )

                for h in range(2):
                    sl = bass.ts(2 * t2 + h, S_TILE)
                    oA = outp.tile([128, S_TILE], f16, tag="oA")
                    nc.scalar.copy(oA[:], ps[2 * h][:])
                    oB = outp.tile([72, S_TILE], f16, tag="oB")
                    nc.scalar.copy(oB[:], ps[2 * h + 1][:])
                    nc.scalar.dma_start(out_d[0:128, sl], oA[:])
                    nc.scalar.dma_start(out_d[128:200, sl], oB[:])

    nc.compile()
    return nc


def kernel(x: np.ndarray, y: np.ndarray, W: np.ndarray) -> np.ndarray:
    from concourse.bass_utils import run_bass_kernel_spmd

    assert x.shape == (BS, DIM, N) and y.shape == (BS, DIM, M)
    assert W.shape == (O, N * M)

    wl = _stage_w(W)
    x_cores = x.reshape(NCORES, S_PER_CORE, N)
    y_cores = y.reshape(NCORES, S_PER_CORE, M)

    in_maps = []
    for i in range(NCORES):
        xil, yrepa, yrepb = _stage_core_inputs(x_cores[i], y_cores[i])
        in_maps.append({"xil": xil, "yrepa": yrepa, "yrepb": yrepb, "wl": wl})

    nc = build_nc()
    res = run_bass_kernel_spmd(nc, in_maps, core_ids=list(range(NCORES)))

    outs = []
    for i in range(NCORES):
        outt = res.results[i]["outt"]  # [O, S_PER_CORE] f16
        outs.append(outt.T.astype(np.float32))  # [S_PER_CORE, O]
    return np.concatenate(outs, axis=0).reshape(BS, DIM, O)


if __name__ == "__main__":
    xs = np.random.randn(BS, DIM, N).astype(np.float32)
    ys = np.random.randn(BS, DIM, M).astype(np.float32)
    Ws = (np.random.randn(O, N * M) * (1.0 / np.sqrt(N * M))).astype(np.float32)
    out = kernel(xs, ys, Ws)
    print(out.shape, out.dtype)



# revision 2
# speedup vs baseline: 1.3068x; 1.3068x over previous
"""Trainium2 Bass kernel for nn_CINLayer: out[b,d,o] = sum_{n,m} x[b,d,n]*y[b,d,m]*W[o,n*M+m].

Strategy (8-core data parallel over batch):
  Per sample s, out[o,s] = sum_k Wl[k,o] * Z[k,s] with Z[k,s] = x[s,n(k)]*y[s,m(k)].
  The contraction k (1600 products) is split into 13 chunks of 128 rows.
  The X-factor tile of each chunk (4 x-rows broadcast to 32 partitions each) is
  HOST-STAGED in replicated layout and DMA'd in (the previous DVE
  stream_shuffle approach saturated the vector engine at ~126us).
  Z chunks are one fp16 tensor_mul each (DVE, a few on GPSIMD), then feed
  fp16 matmuls accumulating out^T[o, s] in PSUM (o split 128+72, s tiles 512).

  Chunk row mapping (r = 32j + r', j=quadrant):
    Part A (c<10):  (n, m) = (4c + j, r')          for r' < 32
    Part B (cb=c-10<3): r' = 8a + m''; (n, m) = (16cb + 4a + j, 32 + m'')
  Host layouts:
    xe[p, t2, c, s'] = xT[n(c,p), t2*1024+s']  (0 where padded)
    yab[p, t2, 0, s'] = yT[p % 32, ...]  (part A), [.,.,1,.] = yT[32 + p%8] (B)
  W rows with n >= 40 (part B overhang) are zeroed on host.
"""

import numpy as np

BS, DIM, N, M, O = 2048, 32, 40, 40, 200
NCORES = 8
S_PER_CORE = BS * DIM // NCORES  # 8192
S_TILE = 512
T2W = 2 * S_TILE  # 1024: samples per inner iteration
NT2 = S_PER_CORE // T2W  # 8
NCHUNKS = 13  # 10 part-A + 3 part-B
F16 = np.float16

# chunks whose Z-multiply runs on GPSIMD instead of DVE (load balancing).
GPSIMD_MULS = frozenset({5, 11})


def _chunk_row_to_nm(c: int, r: int):
    """Global chunk c (0..12), row r (0..127) -> (n, m) or None (zero pad)."""
    j, rp = divmod(r, 32)
    if c < 10:
        return 4 * c + j, rp
    cb = c - 10
    a, mpp = divmod(rp, 8)
    n = 16 * cb + 4 * a + j
    if n >= N:
        return None
    return n, 32 + mpp


def _n_index():
    """[NCHUNKS, 128] x-row index per (chunk, partition), -1 for pad."""
    idx = np.full((NCHUNKS, 128), -1, dtype=np.int64)
    for c in range(NCHUNKS):
        for r in range(128):
            nm = _chunk_row_to_nm(c, r)
            if nm is not None:
                idx[c, r] = nm[0]
    return idx


_N_IDX = _n_index()


def _stage_w(W: np.ndarray) -> np.ndarray:
    """W [O, N*M] f32 -> wl [128, NCHUNKS, O] f16 (lhsT layout per chunk)."""
    Wr = W.reshape(O, N, M)
    wl = np.zeros((128, NCHUNKS, O), dtype=F16)
    for c in range(NCHUNKS):
        for r in range(128):
            nm = _chunk_row_to_nm(c, r)
            if nm is not None:
                wl[r, c, :] = Wr[:, nm[0], nm[1]].astype(F16)
    return wl


def _stage_core_inputs(x_flat: np.ndarray, y_flat: np.ndarray):
    """[S_PER_CORE, 40] f32 x2 -> xe [128, NT2, 13, T2W], yab [128, NT2, 2, T2W] f16."""
    xT = np.ascontiguousarray(x_flat.T).astype(F16)  # [40, S]
    yT = np.ascontiguousarray(y_flat.T).astype(F16)  # [40, S]
    s = xT.shape[1]
    xe = xT[np.clip(_N_IDX, 0, None)]  # [13, 128, S]
    xe[_N_IDX < 0] = 0
    xe = np.ascontiguousarray(
        xe.reshape(NCHUNKS, 128, NT2, T2W).transpose(1, 2, 0, 3)
    )  # [128, NT2, 13, T2W]
    ya = yT[np.arange(128) % 32]  # [128, S]
    yb = yT[32 + (np.arange(128) % 8)]
    yab = np.ascontiguousarray(
        np.stack([ya, yb], axis=1).reshape(128, 2, NT2, T2W).transpose(0, 2, 1, 3)
    )  # [128, NT2, 2, T2W]
    return xe, yab


def _stage_all(x: np.ndarray, y: np.ndarray, W: np.ndarray):
    wl = _stage_w(W)
    x_cores = x.reshape(NCORES, S_PER_CORE, N)
    y_cores = y.reshape(NCORES, S_PER_CORE, M)
    in_maps = []
    for i in range(NCORES):
        xe, yab = _stage_core_inputs(x_cores[i], y_cores[i])
        in_maps.append({"xe": xe, "yab": yab, "wl": wl})
    return in_maps


def build_nc(n_t2: int = NT2, debug: bool = False):
    """Build the per-core Bass/Tile module. Returns nc."""
    import concourse.bass as bass
    import concourse.tile as tile
    from concourse import bacc, mybir

    f16 = mybir.dt.float16
    f32 = mybir.dt.float32
    s_len = n_t2 * T2W

    nc = bacc.Bacc("TRN2", target_bir_lowering=False, debug=debug)

    xe_d = nc.dram_tensor("xe", [128, n_t2, NCHUNKS, T2W], f16, kind="ExternalInput")
    yab_d = nc.dram_tensor("yab", [128, n_t2, 2, T2W], f16, kind="ExternalInput")
    wl_d = nc.dram_tensor("wl", [128, NCHUNKS, O], f16, kind="ExternalInput")
    out_d = nc.dram_tensor("outt", [O, s_len], f16, kind="ExternalOutput")

    with tile.TileContext(nc) as tc:
        with (
            tc.tile_pool(name="wpool", bufs=1) as wpool,
            tc.tile_pool(name="inp", bufs=2) as inp,
            tc.tile_pool(name="zp", bufs=8) as zp,
            tc.tile_pool(name="outp", bufs=4) as outp,
            tc.tile_pool(name="ps", bufs=2, space=bass.MemorySpace.PSUM) as psp,
        ):
            wl_sb = wpool.tile([128, NCHUNKS, O], f16)
            nc.sync.dma_start(wl_sb[:], wl_d[:])

            for t2 in range(n_t2):
                xet = inp.tile([128, NCHUNKS, T2W], f16, tag="xet")
                nc.sync.dma_start(xet[:], xe_d[:, t2])
                yt = inp.tile([128, 2, T2W], f16, tag="yt")
                nc.sync.dma_start(yt[:], yab_d[:, t2])

                psA0 = psp.tile([128, S_TILE], f32, tag="psA0")
                psB0 = psp.tile([72, S_TILE], f32, tag="psB0")
                psA1 = psp.tile([128, S_TILE], f32, tag="psA1")
                psB1 = psp.tile([72, S_TILE], f32, tag="psB1")
                ps = [psA0, psB0, psA1, psB1]
                for c in range(NCHUNKS):
                    z = zp.tile([128, T2W], f16)
                    ysl = yt[:, 0 if c < 10 else 1, :]
                    eng = nc.gpsimd if c in GPSIMD_MULS else nc.vector
                    eng.tensor_mul(z[:], ysl, xet[:, c, :])
                    first, last = c == 0, c == NCHUNKS - 1
                    for h in range(2):
                        zh = z[:, h * S_TILE : (h + 1) * S_TILE]
                        nc.tensor.matmul(
                            ps[2 * h][:], wl_sb[:, c, 0:128], zh,
                            start=first, stop=last,
                        )
                        nc.tensor.matmul(
                            ps[2 * h + 1][:], wl_sb[:, c, 128:200], zh,
                            start=first, stop=last,
                        )

                for h in range(2):
                    sl = bass.ts(2 * t2 + h, S_TILE)
                    oA = outp.tile([128, S_TILE], f16, tag="oA")
                    nc.scalar.copy(oA[:], ps[2 * h][:])
                    oB = outp.tile([72, S_TILE], f16, tag="oB")
                    nc.scalar.copy(oB[:], ps[2 * h + 1][:])
                    nc.scalar.dma_start(out_d[0:128, sl], oA[:])
                    nc.scalar.dma_start(out_d[128:200, sl], oB[:])

    nc.compile()
    return nc


def kernel(x: np.ndarray, y: np.ndarray, W: np.ndarray) -> np.ndarray:
    from concourse.bass_utils import run_bass_kernel_spmd

    assert x.shape == (BS, DIM, N) and y.shape == (BS, DIM, M)
    assert W.shape == (O, N * M)

    in_maps = _stage_all(x, y, W)
    nc = build_nc()
    res = run_bass_kernel_spmd(nc, in_maps, core_ids=list(range(NCORES)))

    outs = []
    for i in range(NCORES):
        outt = res.results[i]["outt"]  # [O, S_PER_CORE] f16
        outs.append(outt.T.astype(np.float32))  # [S_PER_CORE, O]
    return np.concatenate(outs, axis=0).reshape(BS, DIM, O)


if __name__ == "__main__":
    xs = np.random.randn(BS, DIM, N).astype(np.float32)
    ys = np.random.randn(BS, DIM, M).astype(np.float32)
    Ws = (np.random.randn(O, N * M) * (1.0 / np.sqrt(N * M))).astype(np.float32)
    out = kernel(xs, ys, Ws)
    print(out.shape, out.dtype)


# revision 5
# speedup vs baseline: 1.4011x; 1.0721x over previous
"""Trainium2 Bass kernel for nn_CINLayer: out[b,d,o] = sum_{n,m} x[b,d,n]*y[b,d,m]*W[o,n*M+m].

Strategy (8-core data parallel over batch):
  Per sample s, out[o,s] = sum_k Wl[k,o] * Z[k,s] with Z[k,s] = x[s,n(k)]*y[s,m(k)].
  The contraction k (1600 products) is split into 13 chunks of 128 rows.
  The X-factor tile of each chunk (4 x-rows broadcast to 32 partitions each) is
  HOST-STAGED in replicated layout and DMA'd in (the previous DVE
  stream_shuffle approach saturated the vector engine at ~126us).
  Z chunks are one fp16 tensor_mul each (DVE, a few on GPSIMD), then feed
  fp16 matmuls accumulating out^T[o, s] in PSUM (o split 128+72, s tiles 512).

  Chunk row mapping (r = 32j + r', j=quadrant):
    Part A (c<10):  (n, m) = (4c + j, r')          for r' < 32
    Part B (cb=c-10<3): r' = 8a + m''; (n, m) = (16cb + 4a + j, 32 + m'')
  Host layouts:
    xe[p, t2, c, s'] = xT[n(c,p), t2*1024+s']  (0 where padded)
    yab[p, t2, 0, s'] = yT[p % 32, ...]  (part A), [.,.,1,.] = yT[32 + p%8] (B)
  W rows with n >= 40 (part B overhang) are zeroed on host.
"""

import numpy as np

BS, DIM, N, M, O = 2048, 32, 40, 40, 200
NCORES = 8
S_PER_CORE = BS * DIM // NCORES  # 8192
S_TILE = 512
T2W = 2 * S_TILE  # 1024: samples per inner iteration
NT2 = S_PER_CORE // T2W  # 8
NCHUNKS = 13  # 10 part-A + 3 part-B
F16 = np.float16

# chunks whose Z-multiply runs on GPSIMD instead of DVE. GPSIMD shares its
# SBUF port with DVE, so offloading is only a win when DVE is oversubscribed;
# with shuffles gone DVE keeps up alone.
GPSIMD_MULS = frozenset()


def _chunk_row_to_nm(c: int, r: int):
    """Global chunk c (0..12), row r (0..127) -> (n, m) or None (zero pad)."""
    j, rp = divmod(r, 32)
    if c < 10:
        return 4 * c + j, rp
    cb = c - 10
    a, mpp = divmod(rp, 8)
    n = 16 * cb + 4 * a + j
    if n >= N:
        return None
    return n, 32 + mpp


def _n_index():
    """[NCHUNKS, 128] x-row index per (chunk, partition), -1 for pad."""
    idx = np.full((NCHUNKS, 128), -1, dtype=np.int64)
    for c in range(NCHUNKS):
        for r in range(128):
            nm = _chunk_row_to_nm(c, r)
            if nm is not None:
                idx[c, r] = nm[0]
    return idx


_N_IDX = _n_index()


def _stage_w(W: np.ndarray) -> np.ndarray:
    """W [O, N*M] f32 -> wl [128, NCHUNKS, O] f16 (lhsT layout per chunk)."""
    Wr = W.reshape(O, N, M)
    wl = np.zeros((128, NCHUNKS, O), dtype=F16)
    for c in range(NCHUNKS):
        for r in range(128):
            nm = _chunk_row_to_nm(c, r)
            if nm is not None:
                wl[r, c, :] = Wr[:, nm[0], nm[1]].astype(F16)
    return wl


def _stage_core_inputs(x_flat: np.ndarray, y_flat: np.ndarray):
    """[S_PER_CORE, 40] f32 x2 -> xe [128, NT2, 13, T2W], yab [128, NT2, 2, T2W] f16."""
    xT = np.ascontiguousarray(x_flat.T).astype(F16)  # [40, S]
    yT = np.ascontiguousarray(y_flat.T).astype(F16)  # [40, S]
    s = xT.shape[1]
    xe = xT[np.clip(_N_IDX, 0, None)]  # [13, 128, S]
    xe[_N_IDX < 0] = 0
    xe = np.ascontiguousarray(
        xe.reshape(NCHUNKS, 128, NT2, T2W).transpose(1, 2, 0, 3)
    )  # [128, NT2, 13, T2W]
    ya = yT[np.arange(128) % 32]  # [128, S]
    yb = yT[32 + (np.arange(128) % 8)]
    yab = np.ascontiguousarray(
        np.stack([ya, yb], axis=1).reshape(128, 2, NT2, T2W).transpose(0, 2, 1, 3)
    )  # [128, NT2, 2, T2W]
    return xe, yab


def _stage_all(x: np.ndarray, y: np.ndarray, W: np.ndarray):
    wl = _stage_w(W)
    x_cores = x.reshape(NCORES, S_PER_CORE, N)
    y_cores = y.reshape(NCORES, S_PER_CORE, M)
    in_maps = []
    for i in range(NCORES):
        xe, yab = _stage_core_inputs(x_cores[i], y_cores[i])
        in_maps.append({"xe": xe, "yab": yab, "wl": wl})
    return in_maps


def build_nc(n_t2: int = NT2, debug: bool = False):
    """Build the per-core Bass/Tile module. Returns nc."""
    import concourse.bass as bass
    import concourse.tile as tile
    from concourse import bacc, mybir

    f16 = mybir.dt.float16
    f32 = mybir.dt.float32
    s_len = n_t2 * T2W

    nc = bacc.Bacc("TRN2", target_bir_lowering=False, debug=debug)

    xe_d = nc.dram_tensor("xe", [128, n_t2, NCHUNKS, T2W], f16, kind="ExternalInput")
    yab_d = nc.dram_tensor("yab", [128, n_t2, 2, T2W], f16, kind="ExternalInput")
    wl_d = nc.dram_tensor("wl", [128, NCHUNKS, O], f16, kind="ExternalInput")
    out_d = nc.dram_tensor("outt", [O, s_len], f16, kind="ExternalOutput")

    with tile.TileContext(nc) as tc:
        with (
            tc.tile_pool(name="wpool", bufs=1) as wpool,
            tc.tile_pool(name="inp0", bufs=1) as inp0,
            tc.tile_pool(name="inp", bufs=3) as inp,
            tc.tile_pool(name="zp", bufs=8) as zp,
            tc.tile_pool(name="outp", bufs=4) as outp,
            tc.tile_pool(name="ps", bufs=2, space=bass.MemorySpace.PSUM) as psp,
        ):
            wl_sb = wpool.tile([128, NCHUNKS, O], f16)
            nc.sync.dma_start(wl_sb[:], wl_d[:])

            for t2 in range(n_t2):
                if t2 == 0:
                    # per-chunk DMAs so chunk 0's matmuls start ~1us in
                    # instead of waiting for the whole 3.3MB slab.
                    xet = inp0.tile([128, NCHUNKS, T2W], f16, tag="xet0")
                    yt = inp0.tile([128, 2, T2W], f16, tag="yt0")
                    nc.sync.dma_start(yt[:, 0], yab_d[:, t2, 0])
                    nc.sync.dma_start(yt[:, 1], yab_d[:, t2, 1])
                    for c in range(NCHUNKS):
                        nc.sync.dma_start(xet[:, c], xe_d[:, t2, c])
                else:
                    xet = inp.tile([128, NCHUNKS, T2W], f16, tag="xet")
                    nc.sync.dma_start(xet[:], xe_d[:, t2])
                    yt = inp.tile([128, 2, T2W], f16, tag="yt")
                    nc.sync.dma_start(yt[:], yab_d[:, t2])

                psA0 = psp.tile([128, S_TILE], f32, tag="psA0")
                psB0 = psp.tile([72, S_TILE], f32, tag="psB0")
                psA1 = psp.tile([128, S_TILE], f32, tag="psA1")
                psB1 = psp.tile([72, S_TILE], f32, tag="psB1")
                ps = [psA0, psB0, psA1, psB1]
                for c in range(NCHUNKS):
                    z = zp.tile([128, T2W], f16)
                    ysl = yt[:, 0 if c < 10 else 1, :]
                    eng = nc.gpsimd if c in GPSIMD_MULS else nc.vector
                    eng.tensor_mul(z[:], ysl, xet[:, c, :])
                    first, last = c == 0, c == NCHUNKS - 1
                    for h in range(2):
                        zh = z[:, h * S_TILE : (h + 1) * S_TILE]
                        nc.tensor.matmul(
                            ps[2 * h][:], wl_sb[:, c, 0:128], zh,
                            start=first, stop=last,
                        )
                        nc.tensor.matmul(
                            ps[2 * h + 1][:], wl_sb[:, c, 128:200], zh,
                            start=first, stop=last,
                        )

                for h in range(2):
                    sl = bass.ts(2 * t2 + h, S_TILE)
                    # split copies across Scalar and Vector so the PSUM banks
                    # drain in parallel (shorter tail + earlier bank reuse).
                    oA = outp.tile([128, S_TILE], f16, tag="oA")
                    nc.scalar.copy(oA[:], ps[2 * h][:])
                    oB = outp.tile([72, S_TILE], f16, tag="oB")
                    nc.vector.tensor_copy(oB[:], ps[2 * h + 1][:])
                    nc.scalar.dma_start(out_d[0:128, sl], oA[:])
                    nc.scalar.dma_start(out_d[128:200, sl], oB[:])

    nc.compile()
    return nc


def kernel(x: np.ndarray, y: np.ndarray, W: np.ndarray) -> np.ndarray:
    from concourse.bass_utils import run_bass_kernel_spmd

    assert x.shape == (BS, DIM, N) and y.shape == (BS, DIM, M)
    assert W.shape == (O, N * M)

    in_maps = _stage_all(x, y, W)
    nc = build_nc()
    res = run_bass_kernel_spmd(nc, in_maps, core_ids=list(range(NCORES)))

    outs = []
    for i in range(NCORES):
        outt = res.results[i]["outt"]  # [O, S_PER_CORE] f16
        outs.append(outt.T.astype(np.float32))  # [S_PER_CORE, O]
    return np.concatenate(outs, axis=0).reshape(BS, DIM, O)


if __name__ == "__main__":
    xs = np.random.randn(BS, DIM, N).astype(np.float32)
    ys = np.random.randn(BS, DIM, M).astype(np.float32)
    Ws = (np.random.randn(O, N * M) * (1.0 / np.sqrt(N * M))).astype(np.float32)
    out = kernel(xs, ys, Ws)
    print(out.shape, out.dtype)
